# revision 26
# baseline (speedup 1.0000x reference)
"""TRN2 Bass kernel for nn_BasicAttention (dense transformer attention block).

Full module: q/k/v projections -> per-head RMSNorm -> RoPE -> causal GQA
attention -> output projection.

Sharding: tensor-parallel over heads across 8 NeuronCores. Each core owns
2 query heads + 1 kv head (GQA group), computes attention for its heads,
and a partial output projection with its 256-row slice of Wo. The partials
are summed on the host (the unshard/all-reduce step).

v5 (measured 394-397us vs 489us for the v2 baseline on the same
device conditions):
- softmax denominator: DVE pair-sum -> DVE tree (2 levels) -> one
  accumulating ones-matmul per 4 kv-pairs, emitted 2 steps late so the
  PE FIFO never waits on the DVE chain (was: one matmul per pair);
- causal trimming of the diagonal score/PV/exp blocks;
- ops/lps psums alternate between two single-bank pools so a new
  attention loop only waits on the previous epilogue's first reader;
- paired Wo matmuls share one weight load; stage copies split
  DVE:ACT, DVE-heavy when the consuming q-block saturates ACT with exp;
- the reciprocal 1/l runs on DVE (reciprocal_approx_fast), keeping the
  epilogue off the ACT exp stream;
- phase-1's last seq-block postprocess rides the first attention
  q-block (qb=2) as its fillers on a padded schedule; seq blocks are
  processed [0..5,7,6] so that final rope only feeds late attention;
- ~20 dummy matmuls on a zeroed tile warm the HAM clock gate during
  the initial DMA wait (real matmuls start at 2.4 GHz, not 1.2);
- first seq-block DMAs are split per-chunk/per-partition-half across
  queues to cut the cold-start data latency;
- PSUM pools are shared between phases (projection accumulators reuse
  the attention score pool); projection psum copies ride the
  otherwise-idle Scalar engine in phase 1.

Self-contained: hardcodes all shapes; only needs /opt/trn_rl_repo (concourse)
on the python path, which is part of the environment.
"""
import sys

if "/opt/trn_rl_repo" not in sys.path:
    sys.path.insert(0, "/opt/trn_rl_repo")

import numpy as np

S = 4096       # sequence length
HID = 2048     # hidden size
H = 16         # query heads
HKV = 8        # kv heads
D = 128        # head dim
THETA = 10000.0
EPS = 1e-6
NCORES = 8
HPC = H // NCORES          # q heads per core = 2
MQKV = HPC * D + 2 * D     # projection cols per core: 256 q + 128 k + 128 v
EBSLOTS = 6                # exp-score slot buffer depth (reuse distance 3)

_CACHE = {}


def _build(s_len):
    """Build the per-core Bass program (same program on all cores; inputs
    differ). Returns the compiled Bacc module."""
    import concourse.bacc as bacc
    import concourse.tile as tile
    from concourse import mybir

    f32 = mybir.dt.float32
    f32r = mybir.dt.float32r
    bf16 = mybir.dt.bfloat16

    n_sb = s_len // 512            # 512-wide seq blocks for projection phase
    n_kchunk = HID // 128          # 16 contraction chunks
    n_kb = s_len // 128            # attention k blocks
    n_qb = s_len // 512            # attention q blocks
    n_nb = HID // 512              # output hidden blocks

    nc = bacc.Bacc("TRN2", target_bir_lowering=False, debug=False)

    hiddenT = nc.dram_tensor("hiddenT", [HID, s_len], bf16, kind="ExternalInput").ap()
    wqkv = nc.dram_tensor("wqkv", [HID, MQKV], bf16, kind="ExternalInput").ap()
    wo = nc.dram_tensor("wo", [HPC * D, HID], bf16, kind="ExternalInput").ap()
    # norm weights etc, one column vector each
    qkw = nc.dram_tensor("qkw", [D, 4], f32, kind="ExternalInput").ap()
    # rope tables; sin is stacked [+sin; -sin] for the partition-offset trick
    cosst = nc.dram_tensor("cosst", [D, s_len], bf16, kind="ExternalInput").ap()
    sinnst = nc.dram_tensor("sinnst", [D, s_len], bf16, kind="ExternalInput").ap()
    identc = nc.dram_tensor("identc", [128, 128], f32r, kind="ExternalInput").ap()
    onesc = nc.dram_tensor("onesc", [128, 128], bf16, kind="ExternalInput").ap()
    pswapc = nc.dram_tensor("pswapc", [128, 128], bf16, kind="ExternalInput").ap()
    out = nc.dram_tensor("out", [s_len, HID], bf16, kind="ExternalOutput").ap()

    with tile.TileContext(nc) as tc, \
         nc.allow_low_precision("bf16 attention: fp32 PSUM accumulation, "
                                "bf16 elementwise; verified vs fp64 reference"):
        with tc.tile_pool(name="const", bufs=1) as const, \
             tc.tile_pool(name="persist", bufs=1) as persist, \
             tc.tile_pool(name="p2s", bufs=2) as p2s, \
             tc.tile_pool(name="oTp", bufs=4) as oTp, \
             tc.tile_pool(name="p3", bufs=3) as p3, \
             tc.tile_pool(name="ebp", bufs=1) as ebp, \
             tc.tile_pool(name="scps_pool", bufs=2, space="PSUM") as scps_pool, \
             tc.tile_pool(name="accA", bufs=1, space="PSUM") as accA, \
             tc.tile_pool(name="accB", bufs=1, space="PSUM") as accB:
            ident_sb = const.tile([128, 128], f32r, name="ident_sb")
            ones_sb = const.tile([128, 128], bf16, name="ones_sb")
            pswap_sb = const.tile([128, 128], bf16, name="pswap_sb")
            qkw_sb = const.tile([128, 4], f32, name="qkw_sb")
            wo_sb = const.tile([128, HPC, HID], bf16, name="wo_sb")

            # preload the one ACT table set holding Ln+Exp+Copy so the
            # compiler's greedy per-function chooser never thrashes sets
            nc.scalar.add_instruction(mybir.InstLoadActFuncSet(
                name=nc.get_next_instruction_name(), act_func_set_id=6,
                ins=[], outs=[]))

            # persistent activations
            qkT = persist.tile([128, 3, s_len], bf16, name="qkT")  # qT h0, qT h1, kT
            v_sb = persist.tile([128, n_kb, 128], bf16, name="v_sb")
            # exp-score slot buffers, one per q head
            ebufs = [
                ebp.tile([128, EBSLOTS, 1024], bf16, name=f"ebuf{h}",
                         tag=f"ebuf{h}")
                for h in range(HPC)
            ]
            cptog = [0]

            # -------------- attention / output-projection machinery --------
            # ops and lps alternate between two single-bank pools each call:
            # a new loop's ops alloc only waits for the PREVIOUS loop's lps
            # reader (the early Ln of its epilogue), not the full ot chain
            acc_flip = [0]

            def attn_loop(qb, h, fillers):
                ops_pool, lps_pool = ((accA, accB), (accB, accA))[acc_flip[0]]
                acc_flip[0] ^= 1
                qs = qb * 512
                npair = 2 * qb + 2
                ops = ops_pool.tile([128, 512], f32, name="ops", tag="acc")
                esums, e2s = {}, {}
                lps_holder = [None]
                lps_mms = []   # (red, group, is_last) delayed 2 steps
                # step sections ordered oldest-dependency-first so no
                # engine's FIFO head waits on freshly-issued work
                for step in range(npair + 3):
                    if step >= 3:
                        p = step - 3
                        kb0 = 2 * p
                        off0 = max(0, kb0 * 128 - qs)
                        off1 = max(0, (kb0 + 1) * 128 - qs)
                        esb = ebufs[h][:, p % EBSLOTS, :]
                        nc.tensor.matmul(ops[:, off0:512], v_sb[:, kb0, :],
                                         esb[:, off0:512],
                                         start=(p == 0), stop=False)
                        nc.tensor.matmul(ops[:, off1:512], v_sb[:, kb0 + 1, :],
                                         esb[:, 512 + off1:1024],
                                         start=False, stop=(p == npair - 1))
                    if lps_mms and lps_mms[0][0] <= step:
                        _, red, grp, is_last = lps_mms.pop(0)
                        if lps_holder[0] is None:
                            lps_holder[0] = lps_pool.tile(
                                [128, 512], f32, name="lps", tag="acc")
                        nc.tensor.matmul(lps_holder[0], ones_sb, red,
                                         start=(grp == 0), stop=is_last)
                    if step >= 1 and step - 1 < npair:
                        # denominator: DVE pair-sum, then a GpSimd tree and
                        # one accumulating ones-matmul per 4 kv-pairs
                        p = step - 1
                        esb = ebufs[h][:, p % EBSLOTS, :]
                        esum = p2s.tile([128, 512], bf16, name="esum",
                                        tag="esum", bufs=3)
                        nc.vector.tensor_add(esum, esb[:, 0:512],
                                             esb[:, 512:1024])
                        esums[p] = esum
                        if p % 2 == 1:
                            e2 = p2s.tile([128, 512], bf16, name="e2",
                                          tag="e2", bufs=2)
                            nc.vector.tensor_add(e2, esums.pop(p - 1),
                                                 esums.pop(p))
                            e2s[p // 2] = e2
                            red = None
                            if p % 4 == 3:
                                e4 = p2s.tile([128, 512], bf16, name="e4",
                                              tag="e4", bufs=2)
                                nc.vector.tensor_add(e4, e2s.pop(p // 2 - 1),
                                                     e2s.pop(p // 2))
                                red = e4
                            elif p == npair - 1:   # ragged tail group of 2
                                red = e2s.pop(p // 2)
                            if red is not None:
                                lps_mms.append((step + 2, red, p // 4,
                                                p == npair - 1))
                    if step < npair:
                        p = step
                        kb0 = 2 * p
                        # causal q-trim for the diagonal pairs: k-cols below
                        # the block's first k are masked anyway, skip them
                        off0 = max(0, kb0 * 128 - qs)
                        off1 = max(0, (kb0 + 1) * 128 - qs)
                        scps = scps_pool.tile([128, 1024], f32,
                                              name="scps", tag="scps")
                        nc.tensor.matmul(
                            scps[:, off0:512],
                            qkT[:, 2, kb0 * 128:(kb0 + 1) * 128],
                            qkT[:, h, qs + off0:qs + 512],
                            start=True, stop=True)
                        nc.tensor.matmul(
                            scps[:, 512 + off1:1024],
                            qkT[:, 2, (kb0 + 1) * 128:(kb0 + 2) * 128],
                            qkT[:, h, qs + off1:qs + 512],
                            start=True, stop=True)
                        esb = ebufs[h][:, p % EBSLOTS, :]
                        nc.scalar.activation(
                            esb[:, off0:1024], scps[:, off0:1024],
                            mybir.ActivationFunctionType.Exp)
                        if p >= 2 * qb:
                            # zero the k>q region of the diagonal pair (also
                            # clears the exp(0)=1 of bank-cleared columns and
                            # the stale slot data of exp-trimmed columns)
                            nc.gpsimd.affine_select(
                                out=esb.rearrange("p (x q) -> p x q", x=2),
                                in_=esb.rearrange("p (x q) -> p x q", x=2),
                                compare_op=mybir.AluOpType.is_ge,
                                fill=0.0,
                                base=qs - kb0 * 128,
                                pattern=[[-128, 2], [1, 512]],
                                channel_multiplier=-1)
                    if fillers:
                        f = fillers.pop(0)
                        if f is not None:
                            f(True)
                while lps_mms:
                    _, red, grp, is_last = lps_mms.pop(0)
                    if lps_holder[0] is None:
                        lps_holder[0] = lps_pool.tile(
                            [128, 512], f32, name="lps", tag="acc")
                    nc.tensor.matmul(lps_holder[0], ones_sb, red,
                                     start=(grp == 0), stop=is_last)
                return ops, lps_holder[0]

            def emit_lfinish(ops, lps):
                # reciprocal on DVE keeps the epilogue off the saturated ACT
                rl = p2s.tile([128, 512], f32, name="rl", tag="rl")
                nc.vector.reciprocal_approx_fast(rl, lps)
                ot = oTp.tile([128, 512], bf16, name="ot", tag="ot")
                nc.vector.tensor_mul(ot, ops, rl)
                return ot

            def make_wo_units(qb, oTt, mps, dve_heavy=False):
                # per (st4, half): an A unit (2 matmuls, h0, one weight load)
                # and a B unit (2 matmuls, h1, one weight load + stage copies)
                state = {}

                def make_A(st4, half):
                    def emit(in_loop):
                        if half == 0:
                            state[(st4, "stg")] = p3.tile(
                                [128, n_nb, 512], bf16, name="stg4",
                                tag="stg4")
                        wA = mps.tile([128, 512], f32, name="wops", tag="mps")
                        wB = mps.tile([128, 512], f32, name="wops", tag="mps")
                        s4 = slice(st4 * 128, (st4 + 1) * 128)
                        nb0 = 2 * half
                        nc.tensor.matmul(wA, oTt[0][:, s4],
                                         wo_sb[:, 0, nb0 * 512:(nb0 + 1) * 512],
                                         start=True, stop=False)
                        nc.tensor.matmul(wB, oTt[0][:, s4],
                                         wo_sb[:, 0, (nb0 + 1) * 512:(nb0 + 2) * 512],
                                         start=True, stop=False)
                        state[(st4, half)] = (wA, wB)
                    return emit

                def make_B(st4, half):
                    def emit(in_loop):
                        wA, wB = state.pop((st4, half))
                        stg4 = state[(st4, "stg")]
                        s4 = slice(st4 * 128, (st4 + 1) * 128)
                        nb0 = 2 * half
                        nc.tensor.matmul(wA, oTt[1][:, s4],
                                         wo_sb[:, 1, nb0 * 512:(nb0 + 1) * 512],
                                         start=False, stop=True)
                        nc.tensor.matmul(wB, oTt[1][:, s4],
                                         wo_sb[:, 1, (nb0 + 1) * 512:(nb0 + 2) * 512],
                                         start=False, stop=True)
                        for nb, w in ((nb0, wA), (nb0 + 1, wB)):
                            # stage copies split DVE:ACT; DVE-heavy when the
                            # consuming loop is a big q-block whose exp
                            # stream saturates ACT (GpSimd cannot read PSUM)
                            act_turn = (cptog[0] % 4 == 3 if dve_heavy
                                        else cptog[0] % 2 == 1)
                            if act_turn:
                                nc.scalar.copy(stg4[:, nb, :], w)
                            else:
                                nc.vector.tensor_copy(stg4[:, nb, :], w)
                            cptog[0] += 1
                        if half == 1:
                            st = qb * 4 + st4
                            nc.sync.dma_start(
                                out[st * 128:(st + 1) * 128, :],
                                stg4.rearrange("p a b -> p (a b)"))
                    return emit

                units = []
                for st4 in range(4):
                    for half in range(2):
                        units.append(make_A(st4, half))
                        units.append(make_B(st4, half))
                return units

            def emit_wo_final(qb, oTt):
                # tail flush: wide 2-bank psums, weight loads shared per head
                for st4 in range(4):
                    st = qb * 4 + st4
                    stsl = slice(st * 128, (st + 1) * 128)
                    s4 = slice(st4 * 128, (st4 + 1) * 128)
                    stg4 = p3.tile([128, n_nb, 512], bf16,
                                   name="stg4", tag="stg4")
                    for half in range(2):
                        wide = scps_pool.tile([128, 1024], f32,
                                              name="scps", tag="scps")
                        for h in range(HPC):
                            for j in range(2):
                                nb = half * 2 + j
                                nbsl = slice(nb * 512, (nb + 1) * 512)
                                nc.tensor.matmul(
                                    wide[:, j * 512:(j + 1) * 512],
                                    oTt[h][:, s4], wo_sb[:, h, nbsl],
                                    start=(h == 0), stop=(h == HPC - 1))
                        dstv = stg4[:, half * 2:half * 2 + 2, :] \
                            .rearrange("p a b -> p (a b)")
                        if half == 0:
                            nc.vector.tensor_copy(dstv, wide)
                        else:
                            nc.scalar.copy(dstv, wide)
                    nc.sync.dma_start(
                        out[stsl, :], stg4.rearrange("p a b -> p (a b)"))

            # ---------------- Phase 1: projections + norm + rope ----------
            with tc.tile_pool(name="p1c", bufs=1) as p1c, \
                 tc.tile_pool(name="p1", bufs=2) as p1, \
                 tc.tile_pool(name="ssps_pool", bufs=1, space="PSUM") as ssps_pool, \
                 tc.tile_pool(name="ptps", bufs=1, space="PSUM") as ptps:
                csz = max(s_len // 4, 512)
                n_cch = s_len // csz
                cos_chunks = [
                    p1c.tile([128, csz], bf16, name=f"cosc{i}", tag=f"cosc{i}")
                    for i in range(n_cch)
                ]
                sinn_chunks = [
                    p1c.tile([128, csz], bf16, name=f"sinnc{i}", tag=f"sinnc{i}")
                    for i in range(n_cch)
                ]
                wqr = wqkv.rearrange("(k p) m -> p k m", p=128)
                hr = hiddenT.rearrange("(a p) s -> p a s", p=128)
                wq_quads = [
                    p1c.tile([128, 4, MQKV], bf16, name=f"wqq{i}", tag=f"wqq{i}")
                    for i in range(4)
                ]
                # first-needed data first: the first hidden quad, then wq
                # quad 0 split in 4 so the first matmul only waits on row 0,
                # then the remaining weight quads and hidden prefetches so
                # sb0 never runs dry; consts follow.
                # warm the HAM clock gate: ~20 dummy matmuls on a zeroed
                # tile run during the initial DMA wait, so the real stream
                # starts at 2.4 GHz instead of ramping from 1.2
                wzero = p1.tile([128, 512], bf16, name="wzero", tag="wzero",
                                bufs=1)
                nc.gpsimd.memzero(wzero)
                wps = scps_pool.tile([128, 1024], f32, name="warmw",
                                     tag="scps")
                for _ in range(20):
                    nc.tensor.matmul(wps[:, 0:512], wzero[:, 0:128], wzero,
                                     start=True, stop=True)

                hT4_pre = {}
                for kq in range(3):
                    t = p1.tile([128, 4, 512], bf16, name="hT4", tag="hT4",
                                bufs=3)
                    if kq == 0:
                        # fine-grained first loads (partition-split across
                        # DMA queues): the first matmul flush only needs
                        # chunk 0 + wq row 0, so get those in ~2.5us
                        nc.sync.dma_start(t[0:64, 0, :], hr[0:64, 0, 0:512])
                        nc.sync.dma_start(t[64:128, 0, :],
                                          hr[64:128, 0, 0:512])
                        nc.sync.dma_start(wq_quads[0][0:64, 0, :],
                                          wqr[0:64, 0, :])
                        nc.sync.dma_start(wq_quads[0][64:128, 0, :],
                                          wqr[64:128, 0, :])
                        nc.sync.dma_start(t[0:64, 1, :], hr[0:64, 1, 0:512])
                        nc.sync.dma_start(t[64:128, 1, :],
                                          hr[64:128, 1, 0:512])
                        nc.sync.dma_start(t[:, 2:4, :], hr[:, 2:4, 0:512])
                        for j in range(1, 4):
                            nc.sync.dma_start(wq_quads[0][:, j, :],
                                              wqr[:, j, :])
                    else:
                        nc.sync.dma_start(t[:, 0:2, :],
                                          hr[:, 4 * kq:4 * kq + 2, 0:512])
                        nc.sync.dma_start(t[:, 2:4, :],
                                          hr[:, 4 * kq + 2:4 * kq + 4, 0:512])
                    hT4_pre[kq] = t
                for q in range(1, 4):
                    for j in range(4):
                        nc.sync.dma_start(wq_quads[q][:, j, :],
                                          wqr[:, 4 * q + j, :])

                deferred = []   # PE ops from the previous block's postprocess

                # process sb7 BEFORE sb6: the final block's deferred rope
                # then writes kT columns only read late in attention (qb>=6),
                # so the first attention block never waits on it
                sb_order = list(range(n_sb - 2)) + [n_sb - 1, n_sb - 2]
                for sbi, sb in enumerate(sb_order):
                    # 4 accumulating psum column groups in 2 wide tiles
                    # (shared with the attention scps pool)
                    projw = [
                        scps_pool.tile([128, 1024], f32, name=f"projw{j}",
                                       tag="scps")
                        for j in range(2)
                    ]
                    projps = [projw[m // 2][:, (m % 2) * 512:(m % 2 + 1) * 512]
                              for m in range(4)]
                    pend = []   # (k, hT4, kk) waiting for their proj matmuls

                    def flush_mm():
                        k0, hT0, kk0 = pend.pop(0)
                        for m in range(4):
                            nc.tensor.matmul(
                                projps[m],
                                wq_quads[k0 // 4][:, k0 % 4,
                                                  m * 128:(m + 1) * 128],
                                hT0[:, kk0, :],
                                start=(k0 == 0), stop=(k0 == n_kchunk - 1))
                        # interleave one deferred PE op from the previous
                        # block's postprocess; by now its inputs are ready
                        if deferred:
                            deferred.pop(0)()

                    for kq in range(4):
                        # one DMA brings 4 contraction chunks of hidden
                        if sbi == 0 and kq in hT4_pre:
                            hT4 = hT4_pre[kq]
                        else:
                            hT4 = p1.tile([128, 4, 512], bf16, name="hT4",
                                          tag="hT4", bufs=3)
                            ssl = slice(sb * 512, (sb + 1) * 512)
                            if sbi == 0:
                                # first block streams per-chunk for latency
                                for j in range(4):
                                    nc.sync.dma_start(
                                        hT4[:, j, :], hr[:, 4 * kq + j, ssl])
                            else:
                                nc.sync.dma_start(
                                    hT4[:, 0:2, :], hr[:, 4 * kq:4 * kq + 2, ssl])
                                nc.sync.dma_start(
                                    hT4[:, 2:4, :],
                                    hr[:, 4 * kq + 2:4 * kq + 4, ssl])
                        if sbi == 0 and kq == 0:
                            # consts needed from the first postprocess on;
                            # issued after the first wq/hidden loads
                            nc.sync.dma_start(ident_sb, identc)
                            nc.sync.dma_start(ones_sb, onesc)
                            nc.sync.dma_start(pswap_sb, pswapc)
                            nc.sync.dma_start(qkw_sb, qkw)
                        # rope-table chunks must be EMITTED before any rope
                        # op that reads them (emission order defines RAW vs
                        # WAR in Tile) -- chunks 0-2 land in sb0 kq1-3, the
                        # rest early in sb1 (first read is at sb6).
                        ci = None
                        if sbi == 0 and 1 <= kq <= 3 and kq - 1 < n_cch:
                            ci = kq - 1
                        elif sbi == 1 and kq + 3 < n_cch:
                            ci = kq + 3
                        if ci is not None:
                            nc.sync.dma_start(cos_chunks[ci],
                                              cosst[:, ci * csz:(ci + 1) * csz])
                            nc.sync.dma_start(sinn_chunks[ci],
                                              sinnst[:, ci * csz:(ci + 1) * csz])
                        for kk in range(4):
                            pend.append((kq * 4 + kk, hT4, kk))
                            if len(pend) >= 3:
                                flush_mm()
                    while pend:
                        flush_mm()

                    # Free the psum banks fast: all copies + squares first.
                    # Everything downstream (stat matmuls, rope) is deferred
                    # into the next block's MM stream so PE never waits.
                    # The two q heads (m=0,1) share norm weight and score
                    # scale, so their stats run as one 1024-wide stream.
                    cpy01 = p1.tile([128, 2, 512], f32, name="cpy01",
                                    tag="cpy01", bufs=2)
                    # one wide ACT copy: Scalar is idle in phase 1 and this
                    # keeps the psum-freeing chain off the busier DVE
                    nc.scalar.copy(cpy01.rearrange("p a b -> p (a b)"),
                                   projw[0])
                    cpy2 = p1.tile([128, 512], f32, name="cpy2", tag="cpy2",
                                   bufs=2)
                    nc.scalar.copy(cpy2, projps[2])
                    # squares on GpSimd from the SBUF copies: keeps the psum
                    # free chain DVE-only and ACT out of the square work.
                    # The LAST block's squares go to DVE instead, so the
                    # GpSimd queue at the phase boundary holds only the
                    # first attention block's affine-selects
                    sq01 = p1.tile([128, 2, 512], bf16, name="sq01",
                                   tag="sq01", bufs=2)
                    sq2 = p1.tile([128, 512], bf16, name="sq2", tag="sq2",
                                  bufs=2)
                    if sbi == n_sb - 1:
                        nc.vector.tensor_mul(sq01, cpy01, cpy01)
                        nc.vector.tensor_mul(sq2, cpy2, cpy2)
                    else:
                        nc.gpsimd.tensor_mul(sq01, cpy01, cpy01)
                        nc.gpsimd.tensor_mul(sq2, cpy2, cpy2)
                    vT = p1.tile([128, 512], f32r, name="vT", tag="vT")
                    nc.vector.tensor_copy(vT, projps[3])

                    raws = {}

                    def emit_stats01(cpy01=cpy01, sq01=sq01,
                                     wide=(sbi == n_sb - 1)):
                        # two 512-wide halves rotating one psum bank (the
                        # freed bank funds the ops/lps alternation pools);
                        # the LAST block runs one wide 1024 chain on a scps
                        # tile instead -- it sits on the phase-boundary
                        # critical path and the halves would serialize
                        raw01 = p1.tile([128, 2, 512], bf16, name="raw01",
                                        tag="raw01")
                        if wide:
                            ssps = scps_pool.tile([128, 1024], f32,
                                                  name="ssw", tag="scps")
                            nc.tensor.matmul(ssps[:, 0:512], ones_sb,
                                             sq01[:, 0, :],
                                             start=True, stop=True)
                            nc.tensor.matmul(ssps[:, 512:1024], ones_sb,
                                             sq01[:, 1, :],
                                             start=True, stop=True)
                            tln = p1.tile([128, 1024], f32, name="tlnw",
                                          tag="tlnw", bufs=1)
                            nc.scalar.activation(
                                tln, ssps, mybir.ActivationFunctionType.Ln,
                                bias=qkw_sb[:, 2:3], scale=1.0 / 128.0)
                            rq = p1.tile([128, 1024], f32, name="rqw",
                                         tag="rqw", bufs=1)
                            nc.scalar.activation(
                                rq, tln, mybir.ActivationFunctionType.Exp,
                                bias=qkw_sb[:, 3:4], scale=-0.5)
                            nc.vector.scalar_tensor_tensor(
                                raw01.rearrange("p a b -> p (a b)"),
                                cpy01.rearrange("p a b -> p (a b)"),
                                qkw_sb[:, 0:1], rq,
                                op0=mybir.AluOpType.mult,
                                op1=mybir.AluOpType.mult)
                            raws[0] = raw01[:, 0, :]
                            raws[1] = raw01[:, 1, :]
                            return
                        for hh in range(2):
                            ssps = ssps_pool.tile([128, 512], f32,
                                                  name="ssps", tag="ssps",
                                                  bufs=1)
                            nc.tensor.matmul(ssps, ones_sb, sq01[:, hh, :],
                                             start=True, stop=True)
                            tln = p1.tile([128, 512], f32, name="tln",
                                          tag="tln")
                            nc.scalar.activation(
                                tln, ssps, mybir.ActivationFunctionType.Ln,
                                bias=qkw_sb[:, 2:3], scale=1.0 / 128.0)
                            rq = p1.tile([128, 512], f32, name="rq", tag="rq")
                            # q heads fold the 1/sqrt(D) score scale in bias
                            nc.scalar.activation(
                                rq, tln, mybir.ActivationFunctionType.Exp,
                                bias=qkw_sb[:, 3:4], scale=-0.5)
                            nc.vector.scalar_tensor_tensor(
                                raw01[:, hh, :], cpy01[:, hh, :],
                                qkw_sb[:, 0:1], rq,
                                op0=mybir.AluOpType.mult,
                                op1=mybir.AluOpType.mult)
                        raws[0] = raw01[:, 0, :]
                        raws[1] = raw01[:, 1, :]

                    def emit_stats2(cpy2=cpy2, sq2=sq2):
                        ssps = ssps_pool.tile([128, 512], f32, name="ssps",
                                              tag="ssps", bufs=1)
                        nc.tensor.matmul(ssps, ones_sb, sq2,
                                         start=True, stop=True)
                        tln = p1.tile([128, 512], f32, name="tln2", tag="tln2")
                        nc.scalar.activation(
                            tln, ssps,
                            mybir.ActivationFunctionType.Ln,
                            bias=qkw_sb[:, 2:3], scale=1.0 / 128.0)
                        rq = p1.tile([128, 512], f32, name="rq2", tag="rq2")
                        nc.scalar.activation(
                            rq, tln, mybir.ActivationFunctionType.Exp,
                            scale=-0.5)
                        raw = p1.tile([128, 512], bf16, name="raw2",
                                      tag="raw2")
                        nc.vector.scalar_tensor_tensor(
                            raw, cpy2, qkw_sb[:, 1:2], rq,
                            op0=mybir.AluOpType.mult,
                            op1=mybir.AluOpType.mult)
                        raws[2] = raw

                    def make_rope(m, sb=sb):
                        def emit_rope():
                            raw = raws[m]
                            sslm = slice(sb * 512, (sb + 1) * 512)
                            ci, co = sb * 512 // csz, (sb * 512) % csz
                            # half-swap via PE permutation matmul; the
                            # elementwise muls/adds all hit the fast DVE
                            # bf16 2x path (a 64-partition DVE op costs
                            # ~4x a full-width one, measured)
                            bsw = ptps.tile([128, 512], f32, name="bsw",
                                            tag="tps")
                            nc.tensor.matmul(bsw, pswap_sb, raw,
                                             start=True, stop=True)
                            ttc = p1.tile([128, 512], bf16, name="ttc",
                                          tag="ttc")
                            nc.vector.tensor_mul(
                                ttc, raw, cos_chunks[ci][:, co:co + 512])
                            tts = p1.tile([128, 512], bf16, name="tts",
                                          tag="tts")
                            nc.vector.tensor_mul(
                                tts, bsw, sinn_chunks[ci][:, co:co + 512])
                            nc.vector.tensor_add(qkT[:, m, sslm], ttc, tts)
                        return emit_rope

                    def emit_v(vT=vT, sb=sb):
                        vps = ptps.tile([128, 512], f32r, name="vps",
                                        tag="tps")
                        for j in range(4):
                            nc.tensor.transpose(
                                vps[:, j * 128:(j + 1) * 128],
                                vT[:, j * 128:(j + 1) * 128], ident_sb)
                        nc.vector.tensor_copy(
                            v_sb[:, 4 * sb:4 * sb + 4, :]
                            .rearrange("p a b -> p (a b)"),
                            vps)

                    deferred.append(emit_stats01)
                    deferred.append(emit_stats2)
                    deferred.append(make_rope(0))
                    deferred.append(make_rope(1))
                    deferred.append(make_rope(2))
                    deferred.append(emit_v)
                    if sbi == n_sb - 1:
                        nc.sync.dma_start(
                            wo_sb, wo.rearrange("(h p) n -> p h n", p=128))

                # the last block's postprocess rides the first attention
                # q-block (qb=0, which only needs early-sequence K/V) as its
                # fillers -- PE flows straight from projection into scores
                wrapped = [(lambda c: (lambda in_loop: c()))(c)
                           for c in deferred]
                # pad so each closure's PE ops enter the FIFO roughly when
                # their ACT/DVE chain inputs are ready (~1.3us per step)
                first_fillers = [None, wrapped[0], None, wrapped[1], None,
                                 wrapped[2], wrapped[3], wrapped[4],
                                 wrapped[5]]
                deferred = []

                # first attention block is qb=2 (not the tiny qb=0): its
                # ~27 matmuls keep PE dense through the drain so HAM never
                # re-throttles at the phase boundary
                ops0, lps0 = attn_loop(2, 0, first_fillers)
                ot0 = emit_lfinish(ops0, lps0)
                ops1, lps1 = attn_loop(2, 1, first_fillers)
                ot1 = emit_lfinish(ops1, lps1)
                while first_fillers:
                    f = first_fillers.pop(0)
                    if f is not None:
                        f(False)

            # -------- Phases 2+3 interleaved: attention + output proj ------
            with tc.tile_pool(name="mps", bufs=2, space="PSUM") as mps:
                fillers = make_wo_units(2, [ot0, ot1], mps)
                # small q-blocks ride mid-sequence; ending on qb7 keeps the
                # tail dense (its 38 steps absorb the last wo units)
                qbs = [3, 0, 4, 1, 5, 6, 7]
                for qb in qbs:
                    ops0, lps0 = attn_loop(qb, 0, fillers)
                    ot0 = emit_lfinish(ops0, lps0)
                    ops1, lps1 = attn_loop(qb, 1, fillers)
                    ot1 = emit_lfinish(ops1, lps1)
                    while fillers:
                        fillers.pop(0)(False)
                    if qb != qbs[-1]:
                        nxt = qbs[qbs.index(qb) + 1]
                        fillers = make_wo_units(qb, [ot0, ot1], mps,
                                                dve_heavy=(nxt >= 6))
                    else:
                        emit_wo_final(qb, [ot0, ot1])

    nc.compile()
    return nc


def _host_inputs(hidden_state, Wq, Wk, Wv, Wo, q_norm_w, k_norm_w, position_ids,
                 s_len):
    """Build the 8 per-core input maps."""
    import ml_dtypes
    bf16 = ml_dtypes.bfloat16

    half = D // 2
    pos = np.asarray(position_ids).astype(np.float64)
    inv_freq = 1.0 / (THETA ** (np.arange(half, dtype=np.float64) / half))
    ang = pos[:, None] * inv_freq[None, :]          # [S, half]
    cosT = np.cos(ang).T.astype(np.float32)         # [half, S]
    sinT = np.sin(ang).T.astype(np.float32)
    cosst = np.concatenate([cosT, cosT], axis=0).astype(bf16)       # [128, S]
    # [-sin; +sin], matching the 64-partition roll of the pswap matmul
    sinnst = np.concatenate([-sinT, sinT], axis=0).astype(bf16)     # [128, S]
    ident = np.eye(128, dtype=np.float32)
    ones = np.ones((128, 128), dtype=bf16)
    pswap = np.roll(np.eye(128), 64, axis=0).astype(bf16)
    hiddenT = np.asarray(hidden_state, dtype=np.float32).T.astype(bf16)
    qw = np.asarray(q_norm_w, dtype=np.float32)
    kw = np.asarray(k_norm_w, dtype=np.float32)
    epsc = np.full(D, EPS, dtype=np.float32)
    nbq = np.full(D, -0.5 * np.log(128.0), dtype=np.float32)
    qkw = np.stack([qw, kw, epsc, nbq], axis=1)     # [D, 4]

    in_maps = []
    for c in range(NCORES):
        wq_sl = Wq[:, c * HPC * D:(c + 1) * HPC * D]
        wk_sl = Wk[:, c * D:(c + 1) * D]
        wv_sl = Wv[:, c * D:(c + 1) * D]
        wqkv = np.concatenate([wq_sl, wk_sl, wv_sl], axis=1).astype(bf16)
        wo_sl = np.ascontiguousarray(
            Wo[c * HPC * D:(c + 1) * HPC * D, :]).astype(bf16)
        in_maps.append({
            "hiddenT": hiddenT,
            "wqkv": wqkv,
            "wo": wo_sl,
            "qkw": qkw,
            "cosst": cosst,
            "sinnst": sinnst,
            "identc": ident,
            "onesc": ones,
            "pswapc": pswap,
        })
    return in_maps


def kernel(hidden_state, Wq, Wk, Wv, Wo, q_norm_w, k_norm_w, position_ids,
           _s_len=None, _trace=False, **_ignored):
    from concourse.bass_utils import run_bass_kernel_spmd

    # accept jax or numpy inputs
    hidden_state = np.asarray(hidden_state)
    Wq, Wk, Wv, Wo = (np.asarray(w) for w in (Wq, Wk, Wv, Wo))
    q_norm_w = np.asarray(q_norm_w)
    k_norm_w = np.asarray(k_norm_w)
    position_ids = np.asarray(position_ids)

    s_len = int(hidden_state.shape[0]) if _s_len is None else _s_len
    if s_len not in _CACHE:
        _CACHE[s_len] = _build(s_len)
    nc = _CACHE[s_len]

    in_maps = _host_inputs(hidden_state, Wq, Wk, Wv, Wo, q_norm_w, k_norm_w,
                           position_ids, s_len)
    res = run_bass_kernel_spmd(nc, in_maps, core_ids=list(range(NCORES)),
                               trace=_trace)
    kernel._last = res
    acc = res.results[0]["out"].astype(np.float32)
    for c in range(1, NCORES):
        acc += res.results[c]["out"].astype(np.float32)
    return acc
